# revision 1
# baseline (speedup 1.0000x reference)
"""GQA attention forward (B=2, S=2048, D=2048, 16 q heads / 4 kv heads, RoPE,
causal) on 8 Trainium2 NeuronCores.

Sharding: core c <-> (batch b = c//4, kv-group g = c%4). Each core computes its
4 query heads + 1 kv head end-to-end, including its row-shard of wo; the host
sums the 4 wo-partials per batch (the "all-reduce after wo" of the tensor
parallel scheme, done at gather time).

Layout tricks:
  - x is passed transposed (d-major) so every matmul contraction dim lands on
    SBUF partitions.
  - wq/wk columns are permuted per head (even dims -> partitions 0..63, odd ->
    64..127) so RoPE becomes plain elementwise DVE math on partition halves.
    The permutation cancels in q.k dot products.
  - scores are built transposed ([t, s]); softmax denominator is a ones-column
    matmul; no max-subtraction (scores are O(5), exp is safe in fp32, and
    softmax is shift-invariant so the result matches the reference exactly).
  - all matmuls run as float32r (full PE rate at >=256 moving dim, ~1e-4 rel).
  - projection and attention are interleaved per s-block so attention consumes
    q/k/v straight out of SBUF; only the attention output spills to DRAM.
"""

import ml_dtypes
import numpy as np

BF = ml_dtypes.bfloat16
B, S, D = 2, 2048, 2048
N_HEADS, N_KV_HEADS, HD = 16, 4, 128
NH = N_HEADS // N_KV_HEADS  # q heads per core = 4
SB = 512                    # s-block (moving dim per matmul)
NSJ = S // SB               # 4 s-blocks
NT = S // HD                # 16 t-tiles (and d-tiles)
SCALE = 1.0 / np.sqrt(HD).astype(np.float32)

_PROG = None  # built once per process


def _build_program():
    import concourse.bacc as bacc
    import concourse.tile as tile
    from concourse import bass_isa, mybir

    F32 = mybir.dt.float32
    F32R = mybir.dt.float32r
    BF16 = mybir.dt.bfloat16
    Exp = mybir.ActivationFunctionType.Exp

    nc = bacc.Bacc("TRN2", target_bir_lowering=False, debug=False)

    xt_d = nc.declare_dram_parameter("xt", [D, S], F32R, isOutput=False)
    wqkv_d = nc.declare_dram_parameter("wqkv", [D, (NH + 2) * HD], F32R, isOutput=False)
    wo_d = nc.declare_dram_parameter("wo", [NH * HD, D], F32R, isOutput=False)
    cost_d = nc.declare_dram_parameter("cost", [HD // 2, S], F32R, isOutput=False)
    sint_d = nc.declare_dram_parameter("sint", [HD // 2, S], F32R, isOutput=False)
    mask_d = nc.declare_dram_parameter("mask", [NH, HD, SB], F32R, isOutput=False)
    ident_d = nc.declare_dram_parameter("ident", [HD, HD], F32R, isOutput=False)
    ones_d = nc.declare_dram_parameter("ones", [HD, HD], F32R, isOutput=False)
    y_d = nc.declare_dram_parameter("y", [S, D], F32, isOutput=True)

    on_spill = nc.dram_tensor("on_spill", [HD, NH, S], F32R)

    NM = NH + 2  # 6 projection column-blocks: k, v, q0..q3
    H2 = HD // 2

    with tile.TileContext(nc) as tc:
        with (
            tc.tile_pool(name="consts", bufs=1) as consts,
            tc.tile_pool(name="persist", bufs=1) as persist,
            tc.tile_pool(name="work", bufs=2) as work,
        ):
            # ---- small constants (tiles; DMAs emitted after critical loads) ----
            masks = consts.tile([HD, NH, SB], F32R, tag="masks")
            ident = consts.tile([HD, HD], F32R, tag="ident")
            ones_sb = consts.tile([HD, HD], F32R, tag="ones_sb")
            ones_col = ones_sb[:, 0:1]

            # ---- persistent SBUF tensors ----
            kt = persist.tile([HD, S], F32R, tag="kt")           # k^T, rope'd
            v_sb = persist.tile([HD, NT, HD], F32R, tag="v_sb")  # v[t, hd] tiles

            xt_r = xt_d[:, :].rearrange("(t p) s -> p t s", p=HD)

            # ===== merged phase: projection + rope + v-transpose + attention ===
            with (
                tc.tile_pool(name="ph1", bufs=1) as ph1,
                tc.tile_pool(name="xts_pool", bufs=3) as xts_pool,
                tc.tile_pool(name="qk_pool", bufs=6) as qk_pool,
                tc.tile_pool(name="ps12", bufs=1, space="PSUM") as ps12,
            ):
                wqkv = ph1.tile([HD, NT, NM * HD], F32R, tag="wqkv")
                wqkv_r = wqkv_d[:, :].rearrange("(t p) m -> p t m", p=HD)
                sj0_quarters = []
                for ck in range(4):
                    nc.scalar.dma_start(
                        out=wqkv[:, ck * 4 : (ck + 1) * 4, :],
                        in_=wqkv_r[:, ck * 4 : (ck + 1) * 4, :],
                    )
                    xq = xts_pool.tile(
                        [HD, NT // 4, SB], F32R, tag="xts", bufs=6, name=f"xts_0_{ck}"
                    )
                    nc.sync.dma_start(out=xq, in_=xt_r[:, ck * 4 : (ck + 1) * 4, 0:SB])
                    sj0_quarters.append(xq)
                cost = ph1.tile([H2, S], F32R, tag="cost")
                sint = ph1.tile([H2, S], F32R, tag="sint")
                nc.scalar.dma_start(out=cost, in_=cost_d[:, :])
                nc.scalar.dma_start(out=sint, in_=sint_d[:, :])
                nc.gpsimd.dma_start(out=masks, in_=mask_d[:, :, :].rearrange("k p s -> p k s"))
                nc.gpsimd.dma_start(out=ident, in_=ident_d[:, :])
                nc.gpsimd.dma_start(out=ones_sb, in_=ones_d[:, :])

                for sj in range(NSJ):
                    s0 = sj * SB
                    # ---- projection of x^T[:, s-block] ----
                    if sj == 0:
                        quarters = sj0_quarters
                    else:
                        quarters = []
                        for ck in range(4):
                            xq = xts_pool.tile(
                                [HD, NT // 4, SB], F32R, tag="xts", bufs=6, name=f"xts_{sj}_{ck}"
                            )
                            nc.sync.dma_start(
                                out=xq, in_=xt_r[:, ck * 4 : (ck + 1) * 4, s0 : s0 + SB]
                            )
                            quarters.append(xq)
                    qk_tiles = []
                    for m in range(NM):  # 0=k, 1=v, 2..5=q heads
                        pp = ps12.tile([HD, SB], F32, tag="pp", bufs=2)
                        for dt in range(NT):
                            nc.tensor.matmul(
                                out=pp,
                                lhsT=wqkv[:, dt, m * HD : (m + 1) * HD],
                                rhs=quarters[dt // 4][:, dt % 4, :],
                                start=(dt == 0),
                                stop=(dt == NT - 1),
                            )
                        if m != 1:
                            # rope: rows 0:64 = even dims (xr), 64:128 = odd (xi)
                            if m == 0:
                                dst = kt[:, s0 : s0 + SB]
                            else:
                                dst = qk_pool.tile(
                                    [HD, SB], F32R, tag="qk_sb", name=f"q_{sj}_{m}"
                                )
                                qk_tiles.append(dst)
                            c = cost[:, s0 : s0 + SB]
                            sn = sint[:, s0 : s0 + SB]
                            ta = work.tile([H2, SB], F32, tag="rope_a")
                            tb = work.tile([H2, SB], F32, tag="rope_b")
                            nc.vector.tensor_mul(out=ta, in0=pp[0:H2, :], in1=c)
                            nc.vector.tensor_mul(out=tb, in0=pp[H2:HD, :], in1=sn)
                            nc.vector.tensor_sub(out=dst[0:H2, :], in0=ta, in1=tb)
                            tc2 = work.tile([H2, SB], F32, tag="rope_a")
                            td = work.tile([H2, SB], F32, tag="rope_b")
                            nc.vector.tensor_mul(out=tc2, in0=pp[0:H2, :], in1=sn)
                            nc.vector.tensor_mul(out=td, in0=pp[H2:HD, :], in1=c)
                            nc.vector.tensor_add(out=dst[H2:HD, :], in0=tc2, in1=td)
                        else:
                            # v: psum holds v^T[hd, s-block]; transpose to v[t, hd]
                            vt_sb = work.tile([HD, SB], F32R, tag="vt_sb")
                            nc.scalar.copy(out=vt_sb, in_=pp)
                            for q in range(SB // HD):
                                pt = ps12.tile([HD, HD], F32R, tag="pp", bufs=2)
                                nc.tensor.transpose(
                                    pt, vt_sb[:, q * HD : (q + 1) * HD], ident
                                )
                                nc.scalar.copy(out=v_sb[:, sj * 4 + q, :], in_=pt)

                    # ---- attention for all heads at this s-block ----
                    nt = 4 * sj + 4  # causal: t-tiles 0..nt-1
                    for h in range(NH):
                        qts = qk_tiles[h]
                        ps_o = ps12.tile([HD, SB], F32, tag="o", bufs=2, name=f"o_{sj}_{h}")
                        ps_den = ps12.tile([1, SB], F32, tag="den", bufs=1, name=f"d_{sj}_{h}")
                        for ti in range(nt):
                            k = ti - 4 * sj
                            c0 = max(0, k) * HD  # first valid column (diag band)
                            ps_s = ps12.tile([HD, SB], F32, tag="s", bufs=3, name=f"s_{sj}_{h}_{ti}")
                            nc.tensor.matmul(
                                out=ps_s[:, c0:SB],
                                lhsT=kt[:, ti * HD : (ti + 1) * HD],
                                rhs=qts[:, c0:SB],
                                start=True,
                                stop=True,
                            )
                            es = work.tile([HD, SB], F32R, tag="es", bufs=6)
                            nc.scalar.activation(
                                out=es[:, c0:SB], in_=ps_s[:, c0:SB], func=Exp,
                                scale=float(SCALE),
                            )
                            if k >= 0:
                                # triangular part: first 128 valid columns
                                nc.vector.tensor_mul(
                                    out=es[:, c0 : c0 + HD],
                                    in0=es[:, c0 : c0 + HD],
                                    in1=masks[:, 0, 0:HD],
                                )
                            nc.tensor.matmul(
                                out=ps_o[:, c0:SB],
                                lhsT=v_sb[:, ti, :],
                                rhs=es[:, c0:SB],
                                start=(ti == 0),
                                stop=(ti == nt - 1),
                            )
                            nc.tensor.matmul(
                                out=ps_den[:, c0:SB],
                                lhsT=ones_col,
                                rhs=es[:, c0:SB],
                                start=(ti == 0),
                                stop=(ti == nt - 1),
                            )
                        # normalize: on = ps_o / den (broadcast over partitions)
                        den_sb = work.tile([1, SB], F32, tag="den_sb")
                        nc.scalar.copy(out=den_sb, in_=ps_den)
                        db = work.tile([HD, SB], F32, tag="db")
                        nc.gpsimd.partition_broadcast(db, den_sb, channels=HD)
                        rb = work.tile([HD, SB], F32, tag="rb")
                        nc.vector.reciprocal_approx_fast(out=rb, in_=db)
                        on_sb = work.tile([HD, SB], F32R, tag="on_sb", bufs=3)
                        nc.vector.tensor_mul(out=on_sb, in0=ps_o, in1=rb)
                        nc.sync.dma_start(out=on_spill[:, h, s0 : s0 + SB], in_=on_sb)

            # ============= phase 3: out = onorm^T @ wo_g =======================
            with (
                tc.tile_pool(name="ph3w", bufs=1) as ph3w,
                tc.tile_pool(name="on3_pool", bufs=3) as on3_pool,
                tc.tile_pool(name="py3", bufs=8, space="PSUM") as py3,
            ):
                wo_sb = ph3w.tile([HD, NH, D], F32R, tag="wo_sb")
                nc.scalar.dma_start(
                    out=wo_sb, in_=wo_d[:, :].rearrange("(h p) d -> p h d", p=HD)
                )
                for st in range(NT):
                    t0 = st * HD
                    on3 = on3_pool.tile([HD, NH, HD], F32R, tag="on3")
                    nc.sync.dma_start(out=on3, in_=on_spill[:, :, t0 : t0 + HD])
                    ps_y = [
                        py3.tile([HD, SB], F32, tag="ps_y", name=f"ps_y_{st}_{dj}")
                        for dj in range(NSJ)
                    ]
                    for hh in range(NH):
                        for dj in range(NSJ):
                            nc.tensor.matmul(
                                out=ps_y[dj],
                                lhsT=on3[:, hh, :],
                                rhs=wo_sb[:, hh, dj * SB : (dj + 1) * SB],
                                start=(hh == 0),
                                stop=(hh == NH - 1),
                            )
                    for dj in range(NSJ):
                        y_sb = work.tile([HD, SB], F32, tag="y_sb", bufs=4)
                        nc.scalar.copy(out=y_sb, in_=ps_y[dj])
                        nc.sync.dma_start(
                            out=y_d[t0 : t0 + HD, dj * SB : (dj + 1) * SB], in_=y_sb
                        )

    nc.compile()
    return nc


def _get_program():
    global _PROG
    if _PROG is None:
        _PROG = _build_program()
    return _PROG


def _make_in_maps(x, freqs_cos, freqs_sin, wq, wk, wv, wo):
    perm = np.concatenate([np.arange(0, HD, 2), np.arange(1, HD, 2)])  # even|odd

    costT = np.ascontiguousarray(freqs_cos.T, dtype=np.float32)  # [64, S]
    sintT = np.ascontiguousarray(freqs_sin.T, dtype=np.float32)

    # diagonal-band causal masks: mask[k][tt, ss] = 1 if tt <= ss - 128*k
    tt = np.arange(HD)[:, None]
    ss = np.arange(SB)[None, :]
    mask = np.stack(
        [(tt <= (ss - HD * k)).astype(np.float32) for k in range(NH)]
    )  # [4, 128, 512]
    ident = np.eye(HD, dtype=np.float32)
    ones = np.ones((HD, HD), dtype=np.float32)

    # permute q/k head-dim columns so rope pairs land on partition halves
    def permute_heads(w, n_heads):
        w = w.reshape(D, n_heads, HD)
        return w[:, :, perm].reshape(D, n_heads * HD)

    wq_p = permute_heads(np.asarray(wq, np.float32), N_HEADS)
    wk_p = permute_heads(np.asarray(wk, np.float32), N_KV_HEADS)
    wv_ = np.asarray(wv, np.float32)
    wo_ = np.asarray(wo, np.float32)
    x_ = np.asarray(x, np.float32)

    in_maps = []
    for c in range(8):
        b, g = divmod(c, 4)
        wqkv = np.concatenate(
            [
                wk_p[:, g * HD : (g + 1) * HD],
                wv_[:, g * HD : (g + 1) * HD],
                wq_p[:, g * NH * HD : (g + 1) * NH * HD],
            ],
            axis=1,
        )
        in_maps.append(
            {
                "xt": np.ascontiguousarray(x_[b].T),
                "wqkv": np.ascontiguousarray(wqkv),
                "wo": np.ascontiguousarray(wo_[g * NH * HD : (g + 1) * NH * HD, :]),
                "cost": costT,
                "sint": sintT,
                "mask": mask,
                "ident": ident,
                "ones": ones,
            }
        )
    return in_maps


def run(x, freqs_cos, freqs_sin, wq, wk, wv, wo, trace=False):
    from concourse.bass_utils import run_bass_kernel_spmd

    nc = _get_program()
    in_maps = _make_in_maps(x, freqs_cos, freqs_sin, wq, wk, wv, wo)
    res = run_bass_kernel_spmd(nc, in_maps, list(range(8)), trace=trace)
    out = np.empty((B, S, D), dtype=np.float32)
    for b in range(B):
        acc = res.results[b * 4]["y"].astype(np.float32).copy()
        for g in range(1, 4):
            acc += res.results[b * 4 + g]["y"]
        out[b] = acc
    return out, res


def kernel(x, freqs_cos, freqs_sin, wq, wk, wv, wo):
    out, _ = run(x, freqs_cos, freqs_sin, wq, wk, wv, wo, trace=False)
    return out



# revision 8
# speedup vs baseline: 1.3055x; 1.3055x over previous
"""GQA attention forward (B=2, S=2048, D=2048, 16 q heads / 4 kv heads, RoPE,
causal) on 8 Trainium2 NeuronCores.

Sharding: core c <-> (batch b = c//4, kv-group g = c%4). Each core computes its
4 query heads + 1 kv head end-to-end, including its row-shard of wo; the host
sums the 4 wo-partials per batch (the "all-reduce after wo" of the tensor
parallel scheme, done at gather time).

Layout tricks:
  - x is passed transposed (d-major) so every matmul contraction dim lands on
    SBUF partitions.
  - wq/wk columns are permuted per head (even dims -> partitions 0..63, odd ->
    64..127) so RoPE becomes plain elementwise DVE math on partition halves.
    The permutation cancels in q.k dot products.
  - all matmuls run in bf16 (2x the fp32r streaming rate on HW); accumulation
    stays fp32 in PSUM. End-to-end rel err ~4e-3 (validated on host).
  - scores are built transposed ([t, s]); the softmax denominator is an
    all-ones-matrix matmul accumulated in PSUM, which lands the denominator
    already broadcast across partitions (no gpsimd broadcast needed).
  - the attention inner loop is software-pipelined (scores/exp run a few
    tiles ahead of the AV/denominator matmuls) so the PE never waits on the
    scalar-engine exp.
  - attention output stays in SBUF; the wo matmuls run per s-block right
    after that block's attention, and y is written out in bf16 (the host
    sums the 4 per-core partials in fp32).
"""

import ml_dtypes
import numpy as np

BF = ml_dtypes.bfloat16
B, S, D = 2, 2048, 2048
N_HEADS, N_KV_HEADS, HD = 16, 4, 128
NH = N_HEADS // N_KV_HEADS  # q heads per core = 4
SB = 512                    # s-block (moving dim per matmul)
NSJ = S // SB               # 4 s-blocks
NT = S // HD                # 16 t-tiles (and d-tiles)
NM = NH + 2                 # 6 projection column-blocks: k, v, q0..q3
H2 = HD // 2
SCALE = 1.0 / np.sqrt(HD).astype(np.float32)

_PROG = None  # built once per process


def _build_program():
    import concourse.bacc as bacc
    import concourse.tile as tile
    from concourse import mybir

    F32 = mybir.dt.float32
    BF16 = mybir.dt.bfloat16
    Exp = mybir.ActivationFunctionType.Exp

    nc = bacc.Bacc("TRN2", target_bir_lowering=False, debug=False)

    xt_d = nc.declare_dram_parameter("xt", [D, S], BF16, isOutput=False)
    wqkv_d = nc.declare_dram_parameter("wqkv", [D, NM * HD], BF16, isOutput=False)
    wo_d = nc.declare_dram_parameter("wo", [NH * HD, D], BF16, isOutput=False)
    cost_d = nc.declare_dram_parameter("cost", [H2, S], F32, isOutput=False)
    sint_d = nc.declare_dram_parameter("sint", [H2, S], F32, isOutput=False)
    tri_d = nc.declare_dram_parameter("tri", [HD, HD], BF16, isOutput=False)
    ident_d = nc.declare_dram_parameter("ident", [HD, HD], BF16, isOutput=False)
    ones_d = nc.declare_dram_parameter("ones", [HD, HD], BF16, isOutput=False)
    y_d = nc.declare_dram_parameter("y", [S, D], BF16, isOutput=True)

    with tile.TileContext(nc) as tc:
        with (
            tc.tile_pool(name="consts", bufs=1) as consts,
            tc.tile_pool(name="persist", bufs=1) as persist,
            tc.tile_pool(name="work", bufs=2) as work,
            tc.tile_pool(name="xts_pool", bufs=1) as xts_pool,
            tc.tile_pool(name="qk_pool", bufs=1) as qk_pool,
            tc.tile_pool(name="es_pool", bufs=1) as es_pool,
            tc.tile_pool(name="ps", bufs=1, space="PSUM") as ps,
        ):
            tri = consts.tile([HD, HD], BF16, tag="tri")
            ident = consts.tile([HD, HD], BF16, tag="ident")
            ones_sb = consts.tile([HD, HD], BF16, tag="ones")
            cost = consts.tile([H2, S], F32, tag="cost")
            sint = consts.tile([H2, S], F32, tag="sint")

            wqkv = persist.tile([HD, NT, NM * HD], BF16, tag="wqkv")
            kt = persist.tile([HD, S], BF16, tag="kt")
            v_sb = persist.tile([HD, NT, HD], BF16, tag="v_sb")
            on_sb = persist.tile([HD, NH, S], BF16, tag="on")
            wo_sb = persist.tile([HD, NH, D], BF16, tag="wo")

            xt_r = xt_d[:, :].rearrange("(t p) s -> p t s", p=HD)
            wqkv_r = wqkv_d[:, :].rearrange("(t p) m -> p t m", p=HD)

            # startup: interleave wqkv + first x block, quarters each
            xts_tiles = {}
            for ck in range(4):
                nc.scalar.dma_start(
                    out=wqkv[:, ck * 4 : (ck + 1) * 4, :],
                    in_=wqkv_r[:, ck * 4 : (ck + 1) * 4, :],
                )
                xq = xts_pool.tile(
                    [HD, NT // 4, SB], BF16, tag="xts", bufs=8, name=f"xts_0_{ck}"
                )
                nc.sync.dma_start(out=xq, in_=xt_r[:, ck * 4 : (ck + 1) * 4, 0:SB])
                xts_tiles[(0, ck)] = xq
            nc.scalar.dma_start(out=cost, in_=cost_d[:, :])
            nc.scalar.dma_start(out=sint, in_=sint_d[:, :])
            nc.gpsimd.dma_start(out=tri, in_=tri_d[:, :])
            nc.gpsimd.dma_start(out=ident, in_=ident_d[:, :])
            nc.gpsimd.dma_start(out=ones_sb, in_=ones_d[:, :])
            nc.scalar.dma_start(
                out=wo_sb, in_=wo_d[:, :].rearrange("(h p) d -> p h d", p=HD)
            )

            for sj in range(NSJ):
                s0 = sj * SB
                if sj > 0:
                    for ck in range(4):
                        xq = xts_pool.tile(
                            [HD, NT // 4, SB], BF16, tag="xts", bufs=8,
                            name=f"xts_{sj}_{ck}",
                        )
                        nc.sync.dma_start(
                            out=xq, in_=xt_r[:, ck * 4 : (ck + 1) * 4, s0 : s0 + SB]
                        )
                        xts_tiles[(sj, ck)] = xq
                quarters = [xts_tiles[(sj, ck)] for ck in range(4)]

                # ---- projection of x^T[:, s-block]: k, v, q0..q3 ----
                q_tiles = []
                for m in range(NM):
                    pp = ps.tile([HD, SB], F32, tag="pp", bufs=2, name=f"pp_{sj}_{m}")
                    for dt in range(NT):
                        nc.tensor.matmul(
                            out=pp,
                            lhsT=wqkv[:, dt, m * HD : (m + 1) * HD],
                            rhs=quarters[dt // 4][:, dt % 4, :],
                            start=(dt == 0),
                            stop=(dt == NT - 1),
                        )
                    if m == 1:
                        # v: psum holds v^T[hd, s-block]; transpose to v[t, hd]
                        vt = work.tile([HD, SB], BF16, tag="vt")
                        nc.scalar.copy(out=vt, in_=pp)
                        for qq in range(SB // HD):
                            pt = ps.tile(
                                [HD, HD], BF16, tag="pp", bufs=2, name=f"pt_{sj}_{qq}"
                            )
                            nc.tensor.transpose(pt, vt[:, qq * HD : (qq + 1) * HD], ident)
                            nc.scalar.copy(out=v_sb[:, sj * 4 + qq, :], in_=pt)
                    else:
                        # rope: rows 0:64 = even dims (xr), 64:128 = odd (xi)
                        # out_even = xr*c - xi*s ; out_odd = xr*s + xi*c
                        # muls on DVE (PSUM input side-steps the same-base-
                        # partition rule); combines on gpsimd (same-base SB)
                        if m == 0:
                            dst = kt[:, s0 : s0 + SB]
                        else:
                            dst = qk_pool.tile(
                                [HD, SB], BF16, tag="qk", bufs=8, name=f"q_{sj}_{m}"
                            )
                            q_tiles.append(dst)
                        c = cost[:, s0 : s0 + SB]
                        sn = sint[:, s0 : s0 + SB]
                        ta = work.tile([H2, SB], F32, tag="ropeA")
                        tb = work.tile([H2, SB], F32, tag="ropeB")
                        nc.vector.tensor_mul(out=ta, in0=pp[0:H2, :], in1=c)
                        nc.vector.tensor_mul(out=tb, in0=pp[H2:HD, :], in1=sn)
                        nc.gpsimd.tensor_sub(out=dst[0:H2, :], in0=ta, in1=tb)
                        tc2 = work.tile([H2, SB], F32, tag="ropeA")
                        td = work.tile([H2, SB], F32, tag="ropeB")
                        nc.vector.tensor_mul(out=tc2, in0=pp[0:H2, :], in1=sn)
                        nc.vector.tensor_mul(out=td, in0=pp[H2:HD, :], in1=c)
                        nc.gpsimd.tensor_add(out=dst[H2:HD, :], in0=tc2, in1=td)

                # ---- attention, software-pipelined over t-tiles ----
                nt = 4 * sj + 4  # causal: t-tiles 0..nt-1
                LOOKAHEAD = 3
                for h in range(NH):
                    qts = q_tiles[h]
                    ps_o = ps.tile([HD, SB], F32, tag="o", bufs=2, name=f"o_{sj}_{h}")
                    ps_den = ps.tile(
                        [HD, SB], F32, tag="den", bufs=1, name=f"den_{sj}_{h}"
                    )

                    def emit_front(ti):
                        kdiag = ti - 4 * sj
                        c0 = max(0, kdiag) * HD  # first valid column (diag band)
                        ps_s = ps.tile(
                            [HD, SB], F32, tag="s", bufs=3, name=f"s_{sj}_{h}_{ti}"
                        )
                        nc.tensor.matmul(
                            out=ps_s[:, c0:SB],
                            lhsT=kt[:, ti * HD : (ti + 1) * HD],
                            rhs=qts[:, c0:SB],
                            start=True,
                            stop=True,
                        )
                        es = es_pool.tile(
                            [HD, SB], BF16, tag="es", bufs=6, name=f"es_{sj}_{h}_{ti}"
                        )
                        nc.scalar.activation(
                            out=es[:, c0:SB], in_=ps_s[:, c0:SB], func=Exp,
                            scale=float(SCALE),
                        )
                        if kdiag >= 0:
                            # triangular part: first HD valid columns
                            nc.gpsimd.tensor_mul(
                                out=es[:, c0 : c0 + HD],
                                in0=es[:, c0 : c0 + HD],
                                in1=tri,
                            )
                        return (ti, es, c0)

                    def emit_back(item):
                        ti, es, c0 = item
                        nc.tensor.matmul(
                            out=ps_o[:, c0:SB],
                            lhsT=v_sb[:, ti, :],
                            rhs=es[:, c0:SB],
                            start=(ti == 0),
                            stop=(ti == nt - 1),
                        )
                        nc.tensor.matmul(
                            out=ps_den[:, c0:SB],
                            lhsT=ones_sb,
                            rhs=es[:, c0:SB],
                            start=(ti == 0),
                            stop=(ti == nt - 1),
                        )

                    pend = []
                    for ti in range(nt):
                        pend.append(emit_front(ti))
                        if len(pend) > LOOKAHEAD:
                            emit_back(pend.pop(0))
                    while pend:
                        emit_back(pend.pop(0))

                    # normalize: on = ps_o * (1/den); den is already broadcast
                    rb = work.tile([HD, SB], F32, tag="rb")
                    nc.vector.reciprocal_approx_fast(out=rb, in_=ps_den)
                    nc.vector.tensor_mul(
                        out=on_sb[:, h, s0 : s0 + SB], in0=ps_o, in1=rb
                    )

                # ---- wo for this s-block's t-tiles ----
                for stl in range(4):
                    st = sj * 4 + stl
                    t0 = st * HD
                    for dj in range(NSJ):
                        ps_y = ps.tile(
                            [HD, SB], F32, tag="pp", bufs=2, name=f"ps_y_{st}_{dj}"
                        )
                        for hh in range(NH):
                            nc.tensor.matmul(
                                out=ps_y,
                                lhsT=on_sb[:, hh, t0 : t0 + HD],
                                rhs=wo_sb[:, hh, dj * SB : (dj + 1) * SB],
                                start=(hh == 0),
                                stop=(hh == NH - 1),
                            )
                        y_sb = work.tile([HD, SB], BF16, tag="ysb", bufs=4)
                        nc.vector.tensor_copy(y_sb, ps_y)
                        nc.gpsimd.dma_start(
                            out=y_d[t0 : t0 + HD, dj * SB : (dj + 1) * SB], in_=y_sb
                        )

    nc.compile()
    return nc


def _get_program():
    global _PROG
    if _PROG is None:
        _PROG = _build_program()
    return _PROG


def _make_in_maps(x, freqs_cos, freqs_sin, wq, wk, wv, wo):
    perm = np.concatenate([np.arange(0, HD, 2), np.arange(1, HD, 2)])  # even|odd

    costT = np.ascontiguousarray(np.asarray(freqs_cos, np.float32).T)  # [64, S]
    sintT = np.ascontiguousarray(np.asarray(freqs_sin, np.float32).T)

    tt = np.arange(HD)[:, None]
    ss = np.arange(HD)[None, :]
    tri = (tt <= ss).astype(BF)  # lower-tri in [t, s]: valid iff t <= s
    ident = np.eye(HD, dtype=BF)
    ones = np.ones((HD, HD), dtype=BF)

    # permute q/k head-dim columns so rope pairs land on partition halves
    def permute_heads(w, n_heads):
        w = np.asarray(w, np.float32).reshape(D, n_heads, HD)
        return w[:, :, perm].reshape(D, n_heads * HD)

    wq_p = permute_heads(wq, N_HEADS)
    wk_p = permute_heads(wk, N_KV_HEADS)
    wv_ = np.asarray(wv, np.float32)
    wo_ = np.asarray(wo, np.float32)
    x_ = np.asarray(x, np.float32)

    in_maps = []
    for c in range(8):
        b, g = divmod(c, 4)
        wqkv = np.concatenate(
            [
                wk_p[:, g * HD : (g + 1) * HD],
                wv_[:, g * HD : (g + 1) * HD],
                wq_p[:, g * NH * HD : (g + 1) * NH * HD],
            ],
            axis=1,
        )
        in_maps.append(
            {
                "xt": np.ascontiguousarray(x_[b].T).astype(BF),
                "wqkv": np.ascontiguousarray(wqkv).astype(BF),
                "wo": np.ascontiguousarray(
                    wo_[g * NH * HD : (g + 1) * NH * HD, :]
                ).astype(BF),
                "cost": costT,
                "sint": sintT,
                "tri": tri,
                "ident": ident,
                "ones": ones,
            }
        )
    return in_maps


def run(x, freqs_cos, freqs_sin, wq, wk, wv, wo, trace=False):
    from concourse.bass_utils import run_bass_kernel_spmd

    nc = _get_program()
    in_maps = _make_in_maps(x, freqs_cos, freqs_sin, wq, wk, wv, wo)
    res = run_bass_kernel_spmd(nc, in_maps, list(range(8)), trace=trace)
    out = np.empty((B, S, D), dtype=np.float32)
    for b in range(B):
        acc = res.results[b * 4]["y"].astype(np.float32)
        for g in range(1, 4):
            acc = acc + res.results[b * 4 + g]["y"].astype(np.float32)
        out[b] = acc
    return out, res


def kernel(x, freqs_cos, freqs_sin, wq, wk, wv, wo):
    out, _ = run(x, freqs_cos, freqs_sin, wq, wk, wv, wo, trace=False)
    return out
